# revision 19
# baseline (speedup 1.0000x reference)
"""LRN (Local Response Normalization, TF-style cross-W+C window) Trainium2 kernel.

Reference computation (on [B,H,W,C] = [32,224,224,64] f32):
    s[b,h,w]   = sum_c x[b,h,w,c]^2
    win[b,h,w] = sum_{d=-5..5} s[b,h,w+d]        (zero-padded SAME over W)
    out        = x / sqrt(1 + 1.0*win)           (bias=1, alpha=1, beta=0.5)

Sharding: pure data-parallel over batch. 8 cores x 4 batches each.
Per-core layout: rows = (b,h) pairs -> 896 rows = 7 tiles of 128 partitions,
free axis = (w, c) = 224*64.

v2 design (bf16 I/O, mixed fp16 internals):
  HBM traffic halves to 51.4 MB/core (the DMA roofline ~150 us at
  358 GB/s/core) and every DVE tensor_tensor runs in 2x_1p mode
  (2 elem/cycle) on 2-byte data.

Per tile [128 rows, 224 w, 64 c] bf16:
  DMA in  (HWDGE, 3.67 MB)
  ACT  Square (4 chunks of 56 w) bf16 -> fp16 x2
  DVE  c-tree:  t32 = x2[0:32]+x2[32:64]  (fp16, 2x)
                t16 = t32[0:16]+t32[16:32]
                t8  = t16[0:8]+t16[8:16]
       reduce_sum(t8, axis=X) -> s_pad[:, 5+.. ] f32   (1x, 8 elem/w)
  DVE  log-shift window adds (f32): w2,w4,w8,t10,win   (11-wide in 5 ops)
  ACT  denom = Sqrt(1*win + 1) f32
  DVE  rstd = reciprocal_approx_fast(denom)            (custom DVE op)
  ACT  rstd_e[128,224,8] bf16 = Copy(rstd bcast)       (stride-0 mid dim)
  DVE  8x tensor_mul on c-chunks of 8: x[:, :, 8k:8k+8] *= rstd_e
       (both operands innermost step 1 -> 2x mode; in-place)
  DMA out (3.67 MB)
"""

import json
import re

import ml_dtypes
import numpy as np

import concourse.bass as bass
import concourse.tile as tile
from concourse import mybir
from concourse.bass_utils import run_bass_kernel_spmd

# Problem constants (hardcoded per harness contract).
B, H, W, C = 32, 224, 224, 64
N_CORES = 8
RADIUS = 5
KWIN = 2 * RADIUS + 1  # 11
BIAS = 1.0
ALPHA = 1.0

P = 128
B_PER_CORE = B // N_CORES          # 4
ROWS = B_PER_CORE * H              # 896
NTILES = ROWS // P                 # 7
N_WCHUNK = 4
WCH = W // N_WCHUNK                # 56
WPAD = W + KWIN - 1                # 234
CEXP = 8                           # c-chunk width for the final multiply

_F32 = mybir.dt.float32
_BF16 = mybir.dt.bfloat16
_F16 = mybir.dt.float16
NP_BF16 = ml_dtypes.bfloat16

# The walrus build in this container accepts only ONE sync-wait slot per TPB
# instruction ("Too many sync wait commands" in setupSyncWait otherwise),
# while Tile's scheduler freely attaches 2-3 waits per instruction. Legalize
# the BIR before compilation: drop same-engine program-order self-waits
# (trivially satisfied on an in-order sequencer) and hoist any remaining
# excess waits onto standalone EventSemaphore instructions just before the
# owning instruction on the same engine.
_ENGINE_SEM = re.compile(r"^(Pool|Activation|PE|DVE|SP)_\d+$")


def _legalize_bir_waits(bir: bytes, max_waits: int = 1) -> bytes:
    d = json.loads(bir)
    incers: dict = {}
    for fn in d["functions"]:
        for bb in fn.get("blocks") or []:
            for ins in bb["instructions"]:
                for u in (ins.get("sync_info") or {}).get("on_update") or []:
                    incers.setdefault(u["id"], set()).add(
                        (ins.get("engine"), ins.get("opcode"))
                    )
    n_ev = 0
    for fn in d["functions"]:
        for bb in fn.get("blocks") or []:
            out = []
            for ins in bb["instructions"]:
                si = ins.get("sync_info")
                waits = (si or {}).get("on_wait") or []
                opcode = ins.get("opcode")
                if (
                    si
                    and len(waits) > max_waits
                    and opcode != "EventSemaphore"
                ):
                    eng = ins.get("engine")
                    kept = []
                    for w in waits:
                        nm = w.get("ant_name", "")
                        srcs = incers.get(w.get("id"), set())
                        if (
                            _ENGINE_SEM.match(nm)
                            and nm.startswith(str(eng) + "_")
                            and srcs
                            and all(
                                e == eng and op != "DMACopy" for e, op in srcs
                            )
                        ):
                            # Same-engine program-order wait: every inc comes
                            # from an earlier instruction on this in-order
                            # engine, so it holds by the time this issues.
                            continue
                        kept.append(w)
                    for w in kept[max_waits:]:
                        n_ev += 1
                        out.append(
                            {
                                "debug": ins.get("debug", 0),
                                "engine": eng,
                                "ins": [],
                                "outs": [],
                                "name": f"evw-{n_ev}",
                                "opcode": "EventSemaphore",
                                "sync_info": {"on_update": [], "on_wait": [w]},
                            }
                        )
                    si["on_wait"] = kept[:max_waits]
                out.append(ins)
            bb["instructions"] = out
    return json.dumps(d).encode()


class _WaitLegalBass(bass.Bass):
    def to_json_bytes(self) -> bytes:
        return _legalize_bir_waits(super().to_json_bytes())


def build_nc(repeats: int = 1) -> bass.Bass:
    """Build the kernel. repeats>1 runs the full LRN pass that many times
    inside one NEFF, each pass with its OWN ExternalInput/ExternalOutput
    DRAM tensors (externally visible I/O can be neither dead-code-eliminated
    nor DMA-forwarded, so every pass does its full HBM traffic). Used only
    for timing: the wall-time slope over `repeats` isolates on-device
    steady-state time per pass from the ~0.5 ms per-call dispatch overhead
    of the axon tunnel."""
    nc = _WaitLegalBass(trn_type="TRN2")
    xs = [
        nc.dram_tensor("x" if r == 0 else f"x{r}", [ROWS, W, C], _BF16,
                       kind="ExternalInput")
        for r in range(repeats)
    ]
    ys = [
        nc.dram_tensor("y" if r == 0 else f"y{r}", [ROWS, W, C], _BF16,
                       kind="ExternalOutput")
        for r in range(repeats)
    ]

    with tile.TileContext(nc) as tc:
        with (
            tc.tile_pool(name="xpool", bufs=3) as xpool,
            tc.tile_pool(name="x2pool", bufs=3) as x2pool,
            tc.tile_pool(name="tpool", bufs=2) as tpool,
            tc.tile_pool(name="spool", bufs=2) as spool,
            tc.tile_pool(name="wpool", bufs=2) as wpool,
            tc.tile_pool(name="epool", bufs=2) as epool,
        ):
            for it in range(NTILES * repeats):
                rep, tix = divmod(it, NTILES)
                r0 = tix * P
                x_tile = xpool.tile([P, W, C], _BF16)
                nc.sync.dma_start(out=x_tile, in_=xs[rep][r0 : r0 + P])

                # c-halves tree: 64 -> 32 -> 16 -> 8 in fp16 (2x mode), then
                # a 1x reduce of the last 8.
                t32 = tpool.tile([P, W, 32], _F16, tag="t32")
                for jc in range(N_WCHUNK):
                    w0 = jc * WCH
                    x2 = x2pool.tile([P, WCH, C], _F16)
                    nc.scalar.activation(
                        out=x2,
                        in_=x_tile[:, w0 : w0 + WCH, :],
                        func=mybir.ActivationFunctionType.Square,
                    )
                    nc.vector.tensor_add(
                        t32[:, w0 : w0 + WCH, :], x2[:, :, 0:32], x2[:, :, 32:64]
                    )
                t16 = tpool.tile([P, W, 16], _F16, tag="t16")
                nc.vector.tensor_add(t16, t32[:, :, 0:16], t32[:, :, 16:32])
                t8 = tpool.tile([P, W, 8], _F16, tag="t8")
                nc.vector.tensor_add(t8, t16[:, :, 0:8], t16[:, :, 8:16])

                # s_pad holds the C-sums with a 5-wide zero border on each side.
                s_pad = spool.tile([P, WPAD], _F32)
                nc.gpsimd.memset(s_pad[:, 0:RADIUS], 0.0)
                nc.gpsimd.memset(s_pad[:, W + RADIUS : WPAD], 0.0)
                nc.vector.reduce_sum(
                    out=s_pad[:, RADIUS : RADIUS + W],
                    in_=t8,
                    axis=mybir.AxisListType.X,
                )

                # Sliding-window sum of width 11 via log-shift composition.
                # win[w] = sum_{d=0..10} s_pad[w+d],  w in [0, 224).
                w2 = wpool.tile([P, WPAD - 1], _F32)  # w2[j] = s[j] + s[j+1]
                nc.vector.tensor_add(w2, s_pad[:, 0 : WPAD - 1], s_pad[:, 1:WPAD])
                w4 = wpool.tile([P, WPAD - 3], _F32)  # covers d 0..3
                nc.vector.tensor_add(w4, w2[:, 0 : WPAD - 3], w2[:, 2 : WPAD - 1])
                w8 = wpool.tile([P, WPAD - 7], _F32)  # covers d 0..7
                nc.vector.tensor_add(w8, w4[:, 0 : WPAD - 7], w4[:, 4 : WPAD - 3])
                t10 = wpool.tile([P, W], _F32)  # d 0..7 plus d 8..9
                nc.vector.tensor_add(t10, w8[:, 0:W], w2[:, 8 : 8 + W])
                win = wpool.tile([P, W], _F32)  # plus d 10
                nc.vector.tensor_add(win, t10, s_pad[:, 10 : 10 + W])

                # denom = sqrt(alpha*win + bias); rstd = 1/denom.
                denom = wpool.tile([P, W], _F32)
                nc.scalar.activation(
                    out=denom,
                    in_=win,
                    func=mybir.ActivationFunctionType.Sqrt,
                    bias=BIAS,
                    scale=ALPHA,
                )
                rstd = wpool.tile([P, W], _F32)
                nc.vector.reciprocal(out=rstd, in_=denom)

                # Expand rstd over a CEXP-wide c-chunk (ACT copy, stride-0
                # inner src dim) so the final multiplies keep innermost
                # step-1 APs on both operands -> DVE 2x mode.
                rstd_e = epool.tile([P, W, CEXP], _BF16)
                rstd_ap = rstd[:, :]
                rstd_bcast = bass.AP(
                    tensor=rstd_ap.tensor,
                    offset=rstd_ap.offset,
                    ap=[rstd_ap.ap[0], rstd_ap.ap[1], [0, CEXP]],
                )
                nc.scalar.activation(
                    out=rstd_e,
                    in_=rstd_bcast,
                    func=mybir.ActivationFunctionType.Copy,
                )

                # out = x * rstd, c-chunks of CEXP, in place.
                for k in range(C // CEXP):
                    xck = x_tile[:, :, k * CEXP : (k + 1) * CEXP]
                    nc.vector.tensor_mul(xck, xck, rstd_e)

                nc.sync.dma_start(out=ys[rep][r0 : r0 + P], in_=x_tile)

    return nc


_NC_CACHE: list = [None]


def _get_nc() -> bass.Bass:
    if _NC_CACHE[0] is None:
        _NC_CACHE[0] = build_nc()
    return _NC_CACHE[0]


def run(x: np.ndarray, **kwargs):
    """Run the SPMD kernel on 8 cores. Returns (out_f32, BassKernelResults)."""
    x = np.ascontiguousarray(x, dtype=np.float32)
    assert x.shape == (B, H, W, C)
    xb = x.astype(NP_BF16)
    nc = _get_nc()
    in_maps = [
        {"x": xb[i * B_PER_CORE : (i + 1) * B_PER_CORE].reshape(ROWS, W, C)}
        for i in range(N_CORES)
    ]
    res = run_bass_kernel_spmd(nc, in_maps, core_ids=list(range(N_CORES)), **kwargs)
    outs = [
        np.asarray(r["y"]).astype(np.float32).reshape(B_PER_CORE, H, W, C)
        for r in res.results
    ]
    out = np.concatenate(outs, axis=0)
    return out, res


def kernel(x: np.ndarray) -> np.ndarray:
    out, _ = run(x)
    return out


def _make_fn(nc):
    """jit-wrapped single bass_exec call for `nc` over the 8-core mesh."""
    import jax
    from jax.sharding import Mesh, PartitionSpec
    from jax.experimental.shard_map import shard_map

    from concourse import bass2jax
    from concourse import mybir as _mybir

    bass2jax.install_neuronx_cc_hook()
    partition_name = (
        nc.partition_id_tensor.name if nc.partition_id_tensor is not None else None
    )
    in_names, out_names, out_avals = [], [], []
    for alloc in nc.m.functions[0].allocations:
        if not isinstance(alloc, _mybir.MemoryLocationSet):
            continue
        name = alloc.memorylocations[0].name
        if alloc.kind == "ExternalInput":
            if name != partition_name:
                in_names.append(name)
        elif alloc.kind == "ExternalOutput":
            out_names.append(name)
            out_avals.append(
                jax.core.ShapedArray(
                    tuple(alloc.tensor_shape), _mybir.dt.np(alloc.dtype)
                )
            )
    all_names = in_names + out_names
    if partition_name is not None:
        all_names = all_names + [partition_name]

    def _body(*args):
        operands = list(args)
        if partition_name is not None:
            operands.append(bass2jax.partition_id_tensor())
        outs = bass2jax._bass_exec_p.bind(
            *operands,
            out_avals=tuple(out_avals),
            in_names=tuple(all_names),
            out_names=tuple(out_names),
            lowering_input_output_aliases=(),
            sim_require_finite=True,
            sim_require_nnan=True,
            nc=nc,
        )
        return tuple(outs)

    devices = jax.devices()[:N_CORES]
    mesh = Mesh(np.asarray(devices), ("core",))
    nspec = len(in_names) + len(out_names)
    fn = jax.jit(
        shard_map(
            _body,
            mesh=mesh,
            in_specs=(PartitionSpec("core"),) * nspec,
            out_specs=(PartitionSpec("core"),) * len(out_names),
            check_rep=False,
        ),
        keep_unused=True,
    )
    return fn, mesh, len(in_names), len(out_names)


def bench(x: np.ndarray, n_rep: int = 17) -> dict:
    """Measure steady-state on-device time per full LRN pass.

    Per-call dispatch through the axon tunnel costs ~0.5 ms and does not
    pipeline, so a cross-call slope cannot resolve sub-ms kernels. Instead
    build the identical kernel with an internal repeat factor R (the full
    x->y pass run R times back to back inside one NEFF; tile-pool slots force
    the same steady-state pipeline as the single-pass kernel) and take the
    slope of single-call wall time between R=1 and R=n_rep. Fixed per-call
    costs (dispatch, kernel preamble/postamble, ACT table loads) cancel in
    the difference; what remains is pure device execution per pass.
    """
    import time

    import jax

    x = np.ascontiguousarray(x, dtype=np.float32)
    fn1, mesh, ni1, no1 = _make_fn(_get_nc())
    fnR, _, niR, noR = _make_fn(build_nc(repeats=n_rep))
    from jax.sharding import PartitionSpec

    xg = x.astype(NP_BF16).reshape(N_CORES * ROWS, W, C)
    sharding = jax.sharding.NamedSharding(mesh, PartitionSpec("core"))
    xd = jax.device_put(xg, sharding)
    zd = jax.device_put(np.zeros_like(xg), sharding)

    args1 = [xd] * ni1 + [zd] * no1
    argsR = [xd] * niR + [zd] * noR

    # Warmup both executables.
    out0 = fn1(*args1)[0]
    jax.block_until_ready(out0)
    jax.block_until_ready(fnR(*argsR)[0])

    def one(fn, args):
        t0 = time.perf_counter()
        jax.block_until_ready(fn(*args)[0])
        return time.perf_counter() - t0

    t1s = [one(fn1, args1) for _ in range(10)]
    tRs = [one(fnR, argsR) for _ in range(10)]
    t1, tR = min(t1s), min(tRs)
    device_ns = (tR - t1) / (n_rep - 1) * 1e9

    result = np.asarray(out0).astype(np.float32).reshape(B, H, W, C)
    return {
        "device_ns": device_ns,
        "t1_ns": t1 * 1e9,
        "tN_ns": tR * 1e9,
        "n_chain": n_rep,
        "out": result,
    }


# revision 26
# speedup vs baseline: 1.2368x; 1.2368x over previous
"""LRN (Local Response Normalization, TF-style cross-W+C window) Trainium2 kernel.

Reference computation (on [B,H,W,C] = [32,224,224,64] f32):
    s[b,h,w]   = sum_c x[b,h,w,c]^2
    win[b,h,w] = sum_{d=-5..5} s[b,h,w+d]        (zero-padded SAME over W)
    out        = x / sqrt(1 + 1.0*win)           (bias=1, alpha=1, beta=0.5)

Sharding: pure data-parallel over batch. 8 cores x 4 batches each.
Per-core layout: rows = (b,h) pairs -> 896 rows = 7 tiles of 128 partitions,
free axis = (w, c) = 224*64.

v2 design (bf16 I/O, mixed fp16 internals):
  HBM traffic halves to 51.4 MB/core (the DMA roofline ~150 us at
  358 GB/s/core) and every DVE tensor_tensor runs in 2x_1p mode
  (2 elem/cycle) on 2-byte data.

Per tile [128 rows, 224 w, 64 c] bf16:
  DMA in  (HWDGE, 3.67 MB)
  ACT  Square (4 chunks of 56 w) bf16 -> fp16 x2
  DVE  c-tree:  t32 = x2[0:32]+x2[32:64]  (fp16, 2x)
                t16 = t32[0:16]+t32[16:32]
                t8  = t16[0:8]+t16[8:16]
       reduce_sum(t8, axis=X) -> s_pad[:, 5+.. ] f32   (1x, 8 elem/w)
  DVE  log-shift window adds (f32): w2,w4,w8,t10,win   (11-wide in 5 ops)
  ACT  denom = Sqrt(1*win + 1) f32
  DVE  rstd = reciprocal_approx_fast(denom)            (custom DVE op)
  ACT  rstd_e[128,224,8] bf16 = Copy(rstd bcast)       (stride-0 mid dim)
  DVE  8x tensor_mul on c-chunks of 8: x[:, :, 8k:8k+8] *= rstd_e
       (both operands innermost step 1 -> 2x mode; in-place)
  DMA out (3.67 MB)
"""

import json
import re

import ml_dtypes
import numpy as np

import concourse.bass as bass
import concourse.tile as tile
from concourse import mybir
from concourse.bass_utils import run_bass_kernel_spmd

# Problem constants (hardcoded per harness contract).
B, H, W, C = 32, 224, 224, 64
N_CORES = 8
RADIUS = 5
KWIN = 2 * RADIUS + 1  # 11
BIAS = 1.0
ALPHA = 1.0

P = 128
B_PER_CORE = B // N_CORES          # 4
ROWS = B_PER_CORE * H              # 896
NTILES = ROWS // P                 # 7
N_WCHUNK = 4
WCH = W // N_WCHUNK                # 56
WPAD = W + KWIN - 1                # 234
CEXP = 8                           # c-chunk width for the final multiply

_F32 = mybir.dt.float32
_BF16 = mybir.dt.bfloat16
_F16 = mybir.dt.float16
NP_BF16 = ml_dtypes.bfloat16

# The walrus build in this container accepts only ONE sync-wait slot per TPB
# instruction ("Too many sync wait commands" in setupSyncWait otherwise),
# while Tile's scheduler freely attaches 2-3 waits per instruction. Legalize
# the BIR before compilation: drop same-engine program-order self-waits
# (trivially satisfied on an in-order sequencer) and hoist any remaining
# excess waits onto standalone EventSemaphore instructions just before the
# owning instruction on the same engine.
_ENGINE_SEM = re.compile(r"^(Pool|Activation|PE|DVE|SP)_\d+$")


def _legalize_bir_waits(bir: bytes, max_waits: int = 1) -> bytes:
    d = json.loads(bir)
    incers: dict = {}
    for fn in d["functions"]:
        for bb in fn.get("blocks") or []:
            for ins in bb["instructions"]:
                for u in (ins.get("sync_info") or {}).get("on_update") or []:
                    incers.setdefault(u["id"], set()).add(
                        (ins.get("engine"), ins.get("opcode"))
                    )
    n_ev = 0
    for fn in d["functions"]:
        for bb in fn.get("blocks") or []:
            out = []
            for ins in bb["instructions"]:
                si = ins.get("sync_info")
                waits = (si or {}).get("on_wait") or []
                opcode = ins.get("opcode")
                if (
                    si
                    and len(waits) > max_waits
                    and opcode != "EventSemaphore"
                ):
                    eng = ins.get("engine")
                    kept = []
                    for w in waits:
                        nm = w.get("ant_name", "")
                        srcs = incers.get(w.get("id"), set())
                        if (
                            _ENGINE_SEM.match(nm)
                            and nm.startswith(str(eng) + "_")
                            and srcs
                            and all(
                                e == eng and op != "DMACopy" for e, op in srcs
                            )
                        ):
                            # Same-engine program-order wait: every inc comes
                            # from an earlier instruction on this in-order
                            # engine, so it holds by the time this issues.
                            continue
                        kept.append(w)
                    for w in kept[max_waits:]:
                        n_ev += 1
                        out.append(
                            {
                                "debug": ins.get("debug", 0),
                                "engine": eng,
                                "ins": [],
                                "outs": [],
                                "name": f"evw-{n_ev}",
                                "opcode": "EventSemaphore",
                                "sync_info": {"on_update": [], "on_wait": [w]},
                            }
                        )
                    si["on_wait"] = kept[:max_waits]
                out.append(ins)
            bb["instructions"] = out
    return json.dumps(d).encode()


class _WaitLegalBass(bass.Bass):
    def to_json_bytes(self) -> bytes:
        return _legalize_bir_waits(super().to_json_bytes())


DEFAULT_CFG = {
    "stage": 4,          # 0=dma, 1=+squares, 2=+tree, 3=+window/rstd, 4=full
    "in_eng": "sync",    # engine issuing the input DMA (HWDGE ring choice)
    "out_eng": "sync",   # engine issuing the output DMA
    "mul": "big_ip",     # chunk_ip | chunk_oop | big_ip | big_oop
    "cexp": CEXP,
    "xbufs": 3,
}


def build_nc(repeats: int = 1, cfg: dict | None = None) -> bass.Bass:
    """Build the kernel. repeats>1 runs the full LRN pass that many times
    inside one NEFF, each pass with its OWN ExternalInput/ExternalOutput
    DRAM tensors (externally visible I/O can be neither dead-code-eliminated
    nor DMA-forwarded, so every pass does its full HBM traffic). Used only
    for timing: the wall-time slope over `repeats` isolates on-device
    steady-state time per pass from the ~0.5 ms per-call dispatch overhead
    of the axon tunnel."""
    cfg = {**DEFAULT_CFG, **(cfg or {})}
    stage = cfg["stage"]
    cexp = cfg["cexp"]
    oop = cfg["mul"].endswith("_oop")
    nc = _WaitLegalBass(trn_type="TRN2")
    xs = [
        nc.dram_tensor("x" if r == 0 else f"x{r}", [ROWS, W, C], _BF16,
                       kind="ExternalInput")
        for r in range(repeats)
    ]
    ys = [
        nc.dram_tensor("y" if r == 0 else f"y{r}", [ROWS, W, C], _BF16,
                       kind="ExternalOutput")
        for r in range(repeats)
    ]

    with tile.TileContext(nc) as tc:
        in_eng = getattr(nc, cfg["in_eng"])
        out_eng = getattr(nc, cfg["out_eng"])
        xbufs = 2 if oop else cfg["xbufs"]
        with (
            tc.tile_pool(name="xpool", bufs=xbufs) as xpool,
            tc.tile_pool(name="ypool", bufs=2) as ypool,
            tc.tile_pool(name="x2pool", bufs=2) as x2pool,
            tc.tile_pool(name="tpool", bufs=2) as tpool,
            tc.tile_pool(name="spool", bufs=2) as spool,
            tc.tile_pool(name="wpool", bufs=2) as wpool,
            tc.tile_pool(name="epool", bufs=2) as epool,
        ):
            for it in range(NTILES * repeats):
                rep, tix = divmod(it, NTILES)
                r0 = tix * P
                x_tile = xpool.tile([P, W, C], _BF16)
                in_eng.dma_start(out=x_tile, in_=xs[rep][r0 : r0 + P])

                if stage == 0:
                    # DMA-only probe.
                    out_eng.dma_start(out=ys[rep][r0 : r0 + P], in_=x_tile)
                    continue
                if stage == 1:
                    # DMA + ACT squares probe (in-place; same ACT cost).
                    for jc in range(N_WCHUNK):
                        w0 = jc * WCH
                        xc = x_tile[:, w0 : w0 + WCH, :]
                        nc.scalar.activation(
                            out=xc, in_=xc,
                            func=mybir.ActivationFunctionType.Square,
                        )
                    out_eng.dma_start(out=ys[rep][r0 : r0 + P], in_=x_tile)
                    continue

                # c-halves tree: 64 -> 32 -> 16 -> 8 in fp16 (2x mode), then
                # a 1x reduce of the last 8.
                t32 = tpool.tile([P, W, 32], _F16, tag="t32")
                for jc in range(N_WCHUNK):
                    w0 = jc * WCH
                    x2 = x2pool.tile([P, WCH, C], _F16)
                    nc.scalar.activation(
                        out=x2,
                        in_=x_tile[:, w0 : w0 + WCH, :],
                        func=mybir.ActivationFunctionType.Square,
                    )
                    nc.vector.tensor_add(
                        t32[:, w0 : w0 + WCH, :], x2[:, :, 0:32], x2[:, :, 32:64]
                    )
                t16 = tpool.tile([P, W, 16], _F16, tag="t16")
                nc.vector.tensor_add(t16, t32[:, :, 0:16], t32[:, :, 16:32])
                t8 = tpool.tile([P, W, 8], _F16, tag="t8")
                nc.vector.tensor_add(t8, t16[:, :, 0:8], t16[:, :, 8:16])

                if stage == 2:
                    nc.vector.tensor_copy(x_tile[:, :, 0:8], t8)
                    out_eng.dma_start(out=ys[rep][r0 : r0 + P], in_=x_tile)
                    continue

                # s_pad holds the C-sums with a 5-wide zero border on each side.
                s_pad = spool.tile([P, WPAD], _F32)
                nc.gpsimd.memset(s_pad[:, 0:RADIUS], 0.0)
                nc.gpsimd.memset(s_pad[:, W + RADIUS : WPAD], 0.0)
                nc.vector.reduce_sum(
                    out=s_pad[:, RADIUS : RADIUS + W],
                    in_=t8,
                    axis=mybir.AxisListType.X,
                )

                # Sliding-window sum of width 11 via log-shift composition.
                # win[w] = sum_{d=0..10} s_pad[w+d],  w in [0, 224).
                w2 = wpool.tile([P, WPAD - 1], _F32)  # w2[j] = s[j] + s[j+1]
                nc.vector.tensor_add(w2, s_pad[:, 0 : WPAD - 1], s_pad[:, 1:WPAD])
                w4 = wpool.tile([P, WPAD - 3], _F32)  # covers d 0..3
                nc.vector.tensor_add(w4, w2[:, 0 : WPAD - 3], w2[:, 2 : WPAD - 1])
                w8 = wpool.tile([P, WPAD - 7], _F32)  # covers d 0..7
                nc.vector.tensor_add(w8, w4[:, 0 : WPAD - 7], w4[:, 4 : WPAD - 3])
                t10 = wpool.tile([P, W], _F32)  # d 0..7 plus d 8..9
                nc.vector.tensor_add(t10, w8[:, 0:W], w2[:, 8 : 8 + W])
                win = wpool.tile([P, W], _F32)  # plus d 10
                nc.vector.tensor_add(win, t10, s_pad[:, 10 : 10 + W])

                # denom = sqrt(alpha*win + bias); rstd = 1/denom.
                denom = wpool.tile([P, W], _F32)
                nc.scalar.activation(
                    out=denom,
                    in_=win,
                    func=mybir.ActivationFunctionType.Sqrt,
                    bias=BIAS,
                    scale=ALPHA,
                )
                rstd = wpool.tile([P, W], _F32)
                nc.vector.reciprocal(out=rstd, in_=denom)

                rstd_ap = rstd[:, :]
                if cfg["mul"].startswith("chunk") or stage == 3:
                    # Expand rstd over a cexp-wide c-chunk (ACT copy, stride-0
                    # inner src dim) so the final multiplies keep innermost
                    # step-1 APs on both operands -> DVE 2x mode.
                    rstd_e = epool.tile([P, W, cexp], _BF16)
                    rstd_bcast = bass.AP(
                        tensor=rstd_ap.tensor,
                        offset=rstd_ap.offset,
                        ap=[rstd_ap.ap[0], rstd_ap.ap[1], [0, cexp]],
                    )
                    nc.scalar.activation(
                        out=rstd_e,
                        in_=rstd_bcast,
                        func=mybir.ActivationFunctionType.Copy,
                    )

                if stage == 3:
                    nc.vector.tensor_copy(x_tile[:, :, 0:cexp], rstd_e)
                    out_eng.dma_start(out=ys[rep][r0 : r0 + P], in_=x_tile)
                    continue

                # out = x * rstd.
                if oop:
                    o_tile = ypool.tile([P, W, C], _BF16)
                else:
                    o_tile = x_tile
                if cfg["mul"].startswith("chunk"):
                    for k in range(C // cexp):
                        nc.vector.tensor_mul(
                            o_tile[:, :, k * cexp : (k + 1) * cexp],
                            x_tile[:, :, k * cexp : (k + 1) * cexp],
                            rstd_e,
                        )
                else:  # big: single instruction, stride-0 broadcast (1x)
                    rstd_big = bass.AP(
                        tensor=rstd_ap.tensor,
                        offset=rstd_ap.offset,
                        ap=[rstd_ap.ap[0], rstd_ap.ap[1], [0, C]],
                    )
                    nc.vector.tensor_mul(o_tile, x_tile, rstd_big)

                out_eng.dma_start(out=ys[rep][r0 : r0 + P], in_=o_tile)

    return nc


_NC_CACHE: list = [None]


def _get_nc() -> bass.Bass:
    if _NC_CACHE[0] is None:
        _NC_CACHE[0] = build_nc()
    return _NC_CACHE[0]


def run(x: np.ndarray, **kwargs):
    """Run the SPMD kernel on 8 cores. Returns (out_f32, BassKernelResults)."""
    x = np.ascontiguousarray(x, dtype=np.float32)
    assert x.shape == (B, H, W, C)
    xb = x.astype(NP_BF16)
    nc = _get_nc()
    in_maps = [
        {"x": xb[i * B_PER_CORE : (i + 1) * B_PER_CORE].reshape(ROWS, W, C)}
        for i in range(N_CORES)
    ]
    res = run_bass_kernel_spmd(nc, in_maps, core_ids=list(range(N_CORES)), **kwargs)
    outs = [
        np.asarray(r["y"]).astype(np.float32).reshape(B_PER_CORE, H, W, C)
        for r in res.results
    ]
    out = np.concatenate(outs, axis=0)
    return out, res


def kernel(x: np.ndarray) -> np.ndarray:
    out, _ = run(x)
    return out


def _make_fn(nc):
    """jit-wrapped single bass_exec call for `nc` over the 8-core mesh."""
    import jax
    from jax.sharding import Mesh, PartitionSpec
    from jax.experimental.shard_map import shard_map

    from concourse import bass2jax
    from concourse import mybir as _mybir

    bass2jax.install_neuronx_cc_hook()
    partition_name = (
        nc.partition_id_tensor.name if nc.partition_id_tensor is not None else None
    )
    in_names, out_names, out_avals = [], [], []
    for alloc in nc.m.functions[0].allocations:
        if not isinstance(alloc, _mybir.MemoryLocationSet):
            continue
        name = alloc.memorylocations[0].name
        if alloc.kind == "ExternalInput":
            if name != partition_name:
                in_names.append(name)
        elif alloc.kind == "ExternalOutput":
            out_names.append(name)
            out_avals.append(
                jax.core.ShapedArray(
                    tuple(alloc.tensor_shape), _mybir.dt.np(alloc.dtype)
                )
            )
    all_names = in_names + out_names
    if partition_name is not None:
        all_names = all_names + [partition_name]

    def _body(*args):
        operands = list(args)
        if partition_name is not None:
            operands.append(bass2jax.partition_id_tensor())
        outs = bass2jax._bass_exec_p.bind(
            *operands,
            out_avals=tuple(out_avals),
            in_names=tuple(all_names),
            out_names=tuple(out_names),
            lowering_input_output_aliases=(),
            sim_require_finite=True,
            sim_require_nnan=True,
            nc=nc,
        )
        return tuple(outs)

    devices = jax.devices()[:N_CORES]
    mesh = Mesh(np.asarray(devices), ("core",))
    nspec = len(in_names) + len(out_names)
    fn = jax.jit(
        shard_map(
            _body,
            mesh=mesh,
            in_specs=(PartitionSpec("core"),) * nspec,
            out_specs=(PartitionSpec("core"),) * len(out_names),
            check_rep=False,
        ),
        keep_unused=True,
    )
    return fn, mesh, len(in_names), len(out_names)


def bench(x: np.ndarray, n_rep: int = 17) -> dict:
    """Measure steady-state on-device time per full LRN pass.

    Per-call dispatch through the axon tunnel costs ~0.5 ms and does not
    pipeline, so a cross-call slope cannot resolve sub-ms kernels. Instead
    build the identical kernel with an internal repeat factor R (the full
    x->y pass run R times back to back inside one NEFF; tile-pool slots force
    the same steady-state pipeline as the single-pass kernel) and take the
    slope of single-call wall time between R=1 and R=n_rep. Fixed per-call
    costs (dispatch, kernel preamble/postamble, ACT table loads) cancel in
    the difference; what remains is pure device execution per pass.
    """
    import time

    import jax

    x = np.ascontiguousarray(x, dtype=np.float32)
    fn1, mesh, ni1, no1 = _make_fn(_get_nc())
    fnR, _, niR, noR = _make_fn(build_nc(repeats=n_rep))
    from jax.sharding import PartitionSpec

    xg = x.astype(NP_BF16).reshape(N_CORES * ROWS, W, C)
    sharding = jax.sharding.NamedSharding(mesh, PartitionSpec("core"))
    xd = jax.device_put(xg, sharding)
    zd = jax.device_put(np.zeros_like(xg), sharding)

    args1 = [xd] * ni1 + [zd] * no1
    argsR = [xd] * niR + [zd] * noR

    # Warmup both executables.
    out0 = fn1(*args1)[0]
    jax.block_until_ready(out0)
    jax.block_until_ready(fnR(*argsR)[0])

    def one(fn, args):
        t0 = time.perf_counter()
        jax.block_until_ready(fn(*args)[0])
        return time.perf_counter() - t0

    t1s = [one(fn1, args1) for _ in range(10)]
    tRs = [one(fnR, argsR) for _ in range(10)]
    t1, tR = min(t1s), min(tRs)
    device_ns = (tR - t1) / (n_rep - 1) * 1e9

    result = np.asarray(out0).astype(np.float32).reshape(B, H, W, C)
    return {
        "device_ns": device_ns,
        "t1_ns": t1 * 1e9,
        "tN_ns": tR * 1e9,
        "n_chain": n_rep,
        "out": result,
    }
